# revision 1
# baseline (speedup 1.0000x reference)
"""CollisionRegularizer loss on 8 Trainium2 cores.

Strategy: every pairwise quantity (dist^2, the 6 scaled rotated-radius
projections, the velocity-approach dot) is a low-rank bilinear form in
per-point features, so they are computed as small-K matmuls on the PE
(host-prepped augmented feature rows), followed by a short elementwise
chain on DVE/ACT. Sharding: core c handles batch c//2, row-half c%2.
Each core emits per-partition partial sums; the host reduces.
"""

import numpy as np

import concourse.bacc as bacc
import concourse.mybir as mybir
from concourse import tile
from concourse.bass_utils import run_bass_kernel_spmd

B, N = 4, 2048
NC = 8
ROWS = 1024          # n-rows per core
NT = ROWS // 128     # 8 partition tiles
MC = 2               # m-chunks per row-tile
CHUNK = N // MC      # 1024 free-dim chain width
F32 = mybir.dt.float32

MM_TYPES = ["d2", "va", "su0", "su1", "su2", "sv0", "sv1", "sv2"]


def _quat_to_rotmat(q):
    qw, qx, qy, qz = q[..., 0], q[..., 1], q[..., 2], q[..., 3]
    R = np.stack(
        [
            1 - 2 * qy**2 - 2 * qz**2, 2 * qx * qy - 2 * qz * qw, 2 * qx * qz + 2 * qy * qw,
            2 * qx * qy + 2 * qz * qw, 1 - 2 * qx**2 - 2 * qz**2, 2 * qy * qz - 2 * qx * qw,
            2 * qx * qz - 2 * qy * qw, 2 * qy * qz + 2 * qx * qw, 1 - 2 * qx**2 - 2 * qy**2,
        ],
        axis=-1,
    )
    return R.reshape(*q.shape[:-1], 3, 3)


def _prep(xyz, scales, rotations, velocities):
    x = xyz.astype(np.float64)
    s = scales.astype(np.float64)
    v = velocities.astype(np.float64)
    R = _quat_to_rotmat(rotations.astype(np.float64))      # (B,N,3,3)
    a = np.einsum("bni,bnij->bnj", x, R)                   # x_n . R[n][:,j]
    c = (v * x).sum(-1)                                    # v_n . x_n
    nrm = (x * x).sum(-1)

    rhs = np.empty((B, 33, N), np.float32)
    rhs[:, 0:3] = x.transpose(0, 2, 1)
    rhs[:, 3] = 1.0
    rhs[:, 4] = nrm
    rhs[:, 5:8] = v.transpose(0, 2, 1)
    rhs[:, 8] = c
    for j in range(3):
        b0 = 9 + 4 * j
        rhs[:, b0:b0 + 3] = (x * s[:, :, j:j + 1]).transpose(0, 2, 1)
        rhs[:, b0 + 3] = s[:, :, j]
    for j in range(3):
        b0 = 21 + 4 * j
        rhs[:, b0:b0 + 3] = R[:, :, :, j].transpose(0, 2, 1)
        rhs[:, b0 + 3] = a[:, :, j]

    lhs = np.zeros((B, 8, 33, N), np.float32)
    lhs[:, 0, 0:3] = (-2.0 * x).transpose(0, 2, 1)
    lhs[:, 0, 3] = nrm + 1e-8
    lhs[:, 0, 4] = 1.0
    lhs[:, 1, 0:3] = v.transpose(0, 2, 1)
    lhs[:, 1, 3] = -c
    lhs[:, 1, 5:8] = x.transpose(0, 2, 1)
    lhs[:, 1, 8] = -1.0
    for j in range(3):
        b0 = 9 + 4 * j
        lhs[:, 2 + j, b0:b0 + 3] = R[:, :, :, j].transpose(0, 2, 1)
        lhs[:, 2 + j, b0 + 3] = -a[:, :, j]
    for j in range(3):
        b0 = 21 + 4 * j
        lhs[:, 5 + j, b0:b0 + 3] = (x * s[:, :, j:j + 1]).transpose(0, 2, 1)
        lhs[:, 5 + j, b0 + 3] = -s[:, :, j]
    return rhs, lhs


_NC_CACHE = {}

# perf config
F32R = True        # reduced-precision single-pass fp32 matmuls on PE
CHAIN_FP16 = True  # fp16 elementwise chain (2x/4x DVE throughput)
CLAMP = 1e-4       # dist^2 floor; keeps duplicates/diagonal harmless and
                   # bounds inv<=100 so every fp16 intermediate stays in range
F16 = mybir.dt.float16

# engine assignment for flexible elementwise ops: "dve" or "pool"
ASSIGN = {
    "add_r1s": "dve", "add_r2s": "dve", "rsum": "dve",
    "t": "dve", "ovp": "dve", "ov": "act", "den": "dve",
    "sqov": "dve", "g": "dve",
}


def _build(reps=1):
    key = (reps, F32R, CHAIN_FP16, tuple(sorted(ASSIGN.items())))
    if key in _NC_CACHE:
        return _NC_CACHE[key]
    CT = F16 if CHAIN_FP16 else F32
    MMT = mybir.dt.float32r if F32R else F32
    AF = mybir.ActivationFunctionType
    nc = bacc.Bacc(None, target_bir_lowering=False, debug=False)

    def _eng(k):
        return nc.gpsimd if ASSIGN[k] == "pool" else nc.vector

    rhs_d = nc.dram_tensor("rhs", [33, N], MMT, kind="ExternalInput")
    lhs_d = nc.dram_tensor("lhs", [8, 33, ROWS], MMT, kind="ExternalInput")
    rhs32_d = nc.dram_tensor("rhs32", [5, N], F32, kind="ExternalInput")
    lhs32_d = nc.dram_tensor("lhs32", [5, ROWS], F32, kind="ExternalInput")
    out_d = nc.dram_tensor("out", [128, 2 * NT * MC], F32, kind="ExternalOutput")

    with tile.TileContext(nc) as tc:
        with (
            tc.tile_pool(name="io", bufs=1) as io,
            tc.tile_pool(name="wk", bufs=3) as wk,
            tc.tile_pool(name="ch", bufs=3) as ch,
            tc.tile_pool(name="ps", bufs=4, space="PSUM") as ps,
        ):
            rhs_s = io.tile([33, N], MMT)
            nc.sync.dma_start(rhs_s[:], rhs_d[:])
            lhs_t = {}
            for ti, name in enumerate(MM_TYPES):
                lhs_t[name] = io.tile([33, ROWS], MMT, name="lhs_" + name)
                nc.sync.dma_start(lhs_t[name][:], lhs_d[ti])
            rhs32_s = io.tile([5, N], F32)
            nc.sync.dma_start(rhs32_s[:], rhs32_d[:])
            lhs32_s = io.tile([5, ROWS], F32)
            nc.sync.dma_start(lhs32_s[:], lhs32_d[:])
            ocols = io.tile([128, 2 * NT * MC], F32)

            from contextlib import nullcontext
            loop_cm = tc.For_i(0, reps, 1) if reps > 1 else nullcontext()
            with loop_cm:
              for nt in range(NT):
                nsl = slice(nt * 128, (nt + 1) * 128)
                for mc in range(MC):
                    it = nt * MC + mc
                    pt = {}
                    for name in MM_TYPES:
                        p = ps.tile([128, CHUNK], F32, name="p_" + name, tag="mm")
                        for h in range(CHUNK // 512):
                            m0 = mc * CHUNK + h * 512
                            if name == "d2":
                                nc.tensor.matmul(
                                    p[:, h * 512:(h + 1) * 512],
                                    lhs32_s[:, nsl],
                                    rhs32_s[:, m0:m0 + 512],
                                    start=True, stop=True,
                                )
                            else:
                                nc.tensor.matmul(
                                    p[:, h * 512:(h + 1) * 512],
                                    lhs_t[name][:, nsl],
                                    rhs_s[:, m0:m0 + 512],
                                    start=True, stop=True,
                                )
                        pt[name] = p

                    # PSUM drains
                    d2c = wk.tile([128, CHUNK], CT)
                    nc.vector.tensor_scalar_max(d2c[:], pt["d2"][:], CLAMP)
                    rva = wk.tile([128, CHUNK], CT)
                    nc.scalar.activation(rva[:], pt["va"][:], AF.Relu, scale=0.1)
                    # r1s via ACT squares (DVE cannot square PSUM) + Pool adds
                    squ = []
                    for j in range(3):
                        sq = wk.tile([128, CHUNK], CT, name=f"squ{j}")
                        nc.scalar.activation(sq[:], pt[f"su{j}"][:], AF.Square)
                        squ.append(sq)
                    r1s = wk.tile([128, CHUNK], CT)
                    _eng("add_r1s").tensor_add(r1s[:], squ[0][:], squ[1][:])
                    _eng("add_r1s").tensor_add(r1s[:], r1s[:], squ[2][:])
                    # r2s via ACT squares + Pool adds
                    sqv = []
                    for j in range(3):
                        sq = wk.tile([128, CHUNK], CT, name=f"sqv{j}")
                        nc.scalar.activation(sq[:], pt[f"sv{j}"][:], AF.Square)
                        sqv.append(sq)
                    r2s = wk.tile([128, CHUNK], CT)
                    _eng("add_r2s").tensor_add(r2s[:], sqv[0][:], sqv[1][:])
                    _eng("add_r2s").tensor_add(r2s[:], r2s[:], sqv[2][:])

                    dist = wk.tile([128, CHUNK], CT)
                    nc.scalar.activation(dist[:], d2c[:], AF.Sqrt)
                    inv = wk.tile([128, CHUNK], CT)
                    with nc.allow_low_precision("fp16 chain: inv<=100, rel err 5e-4"):
                        nc.vector.reciprocal(inv[:], dist[:])
                    r1 = ch.tile([128, CHUNK], CT)
                    nc.scalar.activation(r1[:], r1s[:], AF.Sqrt)
                    r2 = ch.tile([128, CHUNK], CT)
                    nc.scalar.activation(r2[:], r2s[:], AF.Sqrt)

                    rsum = ch.tile([128, CHUNK], CT)
                    _eng("rsum").tensor_add(rsum[:], r1[:], r2[:])
                    t = ch.tile([128, CHUNK], CT)
                    _eng("t").tensor_mul(t[:], rsum[:], inv[:])
                    ovp = ch.tile([128, CHUNK], CT)
                    _eng("ovp").tensor_sub(ovp[:], t[:], dist[:])
                    ov = wk.tile([128, CHUNK], CT)
                    if ASSIGN["ov"] == "act":
                        nc.scalar.activation(ov[:], ovp[:], AF.Relu)
                    else:
                        _eng("ov").tensor_scalar_max(ov[:], ovp[:], 0.0)

                    den = ch.tile([128, CHUNK], CT)
                    if ASSIGN["den"] == "act":
                        nc.scalar.activation(den[:], ov[:], AF.Identity,
                                             bias=1.0, scale=0.1)
                    else:
                        _eng("den").tensor_scalar(den[:], ov[:], 0.1, 1.0,
                                                  mybir.AluOpType.mult,
                                                  mybir.AluOpType.add)
                    rden = ch.tile([128, CHUNK], CT)
                    with nc.allow_low_precision("fp16 chain"):
                        nc.vector.reciprocal(rden[:], den[:])
                    sqov = ch.tile([128, CHUNK], CT)
                    if ASSIGN["sqov"] == "act":
                        nc.scalar.activation(sqov[:], ov[:], AF.Square)
                    else:
                        _eng("sqov").tensor_mul(sqov[:], ov[:], ov[:])
                    spec = ch.tile([128, CHUNK], CT)
                    nc.vector.scalar_tensor_tensor(
                        out=spec[:], in0=sqov[:], scalar=1.0, in1=rden[:],
                        op0=mybir.AluOpType.mult, op1=mybir.AluOpType.mult,
                        accum_out=ocols[:, 2 * it:2 * it + 1])

                    g = ch.tile([128, CHUNK], CT)
                    _eng("g").tensor_mul(g[:], ov[:], inv[:])
                    vt = ch.tile([128, CHUNK], CT)
                    nc.vector.scalar_tensor_tensor(
                        out=vt[:], in0=g[:], scalar=1.0, in1=rva[:],
                        op0=mybir.AluOpType.mult, op1=mybir.AluOpType.mult,
                        accum_out=ocols[:, 2 * it + 1:2 * it + 2])

            nc.sync.dma_start(out_d[:], ocols[:])

    nc.compile()
    _NC_CACHE[key] = nc
    return nc


def make_in_maps(xyz, scales, rotations, velocities):
    rhs, lhs = _prep(xyz, scales, rotations, velocities)
    in_maps = []
    for c in range(NC):
        b, half = c // 2, c % 2
        in_maps.append({
            "rhs": np.ascontiguousarray(rhs[b]),
            "lhs": np.ascontiguousarray(lhs[b][:, :, half * ROWS:(half + 1) * ROWS]),
            "rhs32": np.ascontiguousarray(rhs[b][0:5]),
            "lhs32": np.ascontiguousarray(lhs[b][0, 0:5, half * ROWS:(half + 1) * ROWS]),
        })
    return in_maps


def finish(results):
    total = 0.0
    for c in range(NC):
        total += results[c]["out"].astype(np.float64).sum()
    return np.float32(total / (B * N * N))


_RUNNER = {}


def _get_runner(reps=1):
    """Cached shard_map-jitted executor (mirrors bass2jax.run_bass_via_pjrt
    multi-core path) so repeated calls skip re-compilation."""
    if reps in _RUNNER:
        return _RUNNER[reps]
    import jax
    from jax.sharding import Mesh, PartitionSpec
    from jax.experimental.shard_map import shard_map
    from concourse import bass2jax

    nc = _build(reps)
    bass2jax.install_neuronx_cc_hook()

    part_name = nc.partition_id_tensor.name if nc.partition_id_tensor else None
    in_names, out_names, out_avals, zero_outs = [], [], [], []
    for alloc in nc.m.functions[0].allocations:
        if not isinstance(alloc, mybir.MemoryLocationSet):
            continue
        name = alloc.memorylocations[0].name
        if alloc.kind == "ExternalInput":
            if name != part_name:
                in_names.append(name)
        elif alloc.kind == "ExternalOutput":
            out_names.append(name)
            shape = tuple(alloc.tensor_shape)
            dtype = mybir.dt.np(alloc.dtype)
            out_avals.append(jax.core.ShapedArray(shape, dtype))
            zero_outs.append(np.zeros(shape, dtype))
    n_params = len(in_names)
    all_names = in_names + out_names
    if part_name is not None:
        all_names = all_names + [part_name]

    def _body(*args):
        operands = list(args)
        if part_name is not None:
            operands.append(bass2jax.partition_id_tensor())
        outs = bass2jax._bass_exec_p.bind(
            *operands,
            out_avals=tuple(out_avals),
            in_names=tuple(all_names),
            out_names=tuple(out_names),
            lowering_input_output_aliases=(),
            sim_require_finite=True,
            sim_require_nnan=True,
            nc=nc,
        )
        return tuple(outs)

    devices = jax.devices()[:NC]
    mesh = Mesh(np.asarray(devices), ("core",))
    n_outs = len(out_names)
    fn = jax.jit(
        shard_map(
            _body, mesh=mesh,
            in_specs=(PartitionSpec("core"),) * (n_params + n_outs),
            out_specs=(PartitionSpec("core"),) * n_outs,
            check_rep=False,
        ),
        donate_argnums=tuple(range(n_params, n_params + n_outs)),
        keep_unused=True,
    )

    def run(in_maps):
        concat_in = [
            np.concatenate([in_maps[c][nm] for c in range(NC)], axis=0)
            for nm in in_names
        ]
        concat_zeros = [
            np.zeros((NC * z.shape[0], *z.shape[1:]), z.dtype) for z in zero_outs
        ]
        out_arrs = fn(*concat_in, *concat_zeros)
        return [
            {nm: np.asarray(out_arrs[i]).reshape(NC, *out_avals[i].shape)[c]
             for i, nm in enumerate(out_names)}
            for c in range(NC)
        ]

    _RUNNER[reps] = run
    return run


def kernel(xyz, scales, rotations, velocities):
    run = _get_runner()
    in_maps = make_in_maps(xyz, scales, rotations, velocities)
    return finish(run(in_maps))


if __name__ == "__main__":
    rng = np.random.default_rng(0)
    ins = {
        "xyz": rng.standard_normal((B, N, 3)).astype(np.float32),
        "scales": rng.random((B, N, 3)).astype(np.float32),
        "rotations": rng.standard_normal((B, N, 4)).astype(np.float32),
        "velocities": rng.standard_normal((B, N, 3)).astype(np.float32),
    }
    print(kernel(**ins))



# revision 11
# speedup vs baseline: 4.2571x; 4.2571x over previous
"""CollisionRegularizer loss on 8 Trainium2 cores — v2.

Each pairwise quantity (d2, -v_rel.diff, r1s, r2s) is ONE small-K bilinear
matmul on the PE (host-prepped feature rows); matmul cost is K-independent,
so squares/adds that the v1 kernel did on ACT/DVE are folded into the PE.
Reciprocals use reciprocal_approx_fast (1 pass) + ACT Sqrt instead of the
6-cycle/elem bit-exact DVE reciprocal. The pair matrix is symmetric in
(n,m), so each 128-row block only processes an 8-block wrap-around column
window (distance 1..8) at weight 2 (distance-8 columns at weight 1), plus
one diagonal-block pass at weight 1 — ~56% of the full elementwise work.
Spectral sum uses x^2/(1+.1x) = 10x - 100 + 1000/(10+x) so only per-row
accumulator columns leave the chip; the host combines in f64.
"""

import numpy as np

import concourse.bacc as bacc
import concourse.mybir as mybir
from concourse import tile
from concourse.bass_utils import run_bass_kernel_spmd

B, N = 4, 2048
NC = 8
NB = 16            # 128-row blocks per batch
RPB = 8            # row-blocks per core
FD = 1024          # chain width (8 column blocks)
W2 = 896           # weight-2 slice width (d 1..7); [W2:FD] is d=8 (weight 1)
EXT = N            # rhs width (half-shift baked on host)
KD, KV, KR = 5, 8, 31
EPS = 1e-4         # r1s/r2s floor (guards matmul rounding below sqrt)
CL = 1e-4 + 1e-8   # d2 additive clamp
NCOL = 48          # 8 main iters * 6
F32 = mybir.dt.float32
F16 = mybir.dt.float16
F32R = mybir.dt.float32r
UPIX = [(0, 0), (0, 1), (0, 2), (1, 1), (1, 2), (2, 2)]

# engine for each flexible chain op: "dve" | "pool" | ("act" where noted)
ASSIGN = {
    "rva": "dve", "rsum": "dve", "mnum": "dve", "ovi": "dve",
    "u": "dve", "ruS": "dve", "m1": "dve", "pv": "dve", "vt": "dve",
}


def _quat_to_rotmat(q):
    qw, qx, qy, qz = q[..., 0], q[..., 1], q[..., 2], q[..., 3]
    R = np.stack(
        [
            1 - 2 * qy**2 - 2 * qz**2, 2 * qx * qy - 2 * qz * qw, 2 * qx * qz + 2 * qy * qw,
            2 * qx * qy + 2 * qz * qw, 1 - 2 * qx**2 - 2 * qz**2, 2 * qy * qz - 2 * qx * qw,
            2 * qx * qz - 2 * qy * qw, 2 * qy * qz + 2 * qx * qw, 1 - 2 * qx**2 - 2 * qy**2,
        ],
        axis=-1,
    )
    return R.reshape(*q.shape[:-1], 3, 3)


def _prep(xyz, scales, rotations, velocities):
    x = xyz.astype(np.float64)
    s2 = scales.astype(np.float64) ** 2
    v = velocities.astype(np.float64)
    R = _quat_to_rotmat(rotations.astype(np.float64))      # (B,N,3,3)
    a = np.einsum("bni,bnij->bnj", x, R)                   # x_n . R_n[:,j]
    c = (v * x).sum(-1)
    nrm = (x * x).sum(-1)
    one = np.ones((B, N))

    def st(rows):
        return np.stack(rows, axis=1)                      # (B,K,N)

    lhs_d2 = st([-2 * x[..., 0], -2 * x[..., 1], -2 * x[..., 2], nrm + CL, one])
    rhs_d2 = st([x[..., 0], x[..., 1], x[..., 2], one, nrm])
    lhs_va = st([v[..., 0], v[..., 1], v[..., 2],
                 x[..., 0], x[..., 1], x[..., 2], c, one])
    rhs_va = st([x[..., 0], x[..., 1], x[..., 2],
                 v[..., 0], v[..., 1], v[..., 2], -one, -c])

    def n_rows(j):    # quadratic in (R_n, a_n)
        rows = [a[..., j] ** 2]
        rows += [-2 * a[..., j] * R[..., i, j] for i in range(3)]
        rows += [R[..., i, j] * R[..., k, j] * (2.0 if i != k else 1.0)
                 for (i, k) in UPIX]
        return rows

    def m_rows(j):    # s2_j * [1, x, x (x) x]
        rows = [s2[..., j]]
        rows += [s2[..., j] * x[..., i] for i in range(3)]
        rows += [s2[..., j] * x[..., i] * x[..., k] for (i, k) in UPIX]
        return rows

    nr = n_rows(0) + n_rows(1) + n_rows(2)
    mr = m_rows(0) + m_rows(1) + m_rows(2)
    lhs_r1 = st(nr + [EPS * one])
    rhs_r1 = st(mr + [one])
    lhs_r2 = st(mr + [EPS * one])
    rhs_r2 = st(nr + [one])

    lhs = {"d2": lhs_d2, "va": lhs_va, "r1": lhs_r1, "r2": lhs_r2}
    rhs = {"d2": rhs_d2, "va": rhs_va, "r1": rhs_r1, "r2": rhs_r2}
    lhs = {k: v.astype(np.float32) for k, v in lhs.items()}
    rhs = {k: v.astype(np.float32) for k, v in rhs.items()}
    return lhs, rhs


_NC_CACHE = {}


def _build(reps=1):
    key = (reps, tuple(sorted(ASSIGN.items())))
    if key in _NC_CACHE:
        return _NC_CACHE[key]
    AF = mybir.ActivationFunctionType
    ALU = mybir.AluOpType
    nc = bacc.Bacc(None, target_bir_lowering=False, debug=False)

    def eng(k):
        return nc.gpsimd if ASSIGN[k] == "pool" else nc.vector

    KS = {"d2": KD, "va": KV, "r1": KR, "r2": KR}
    MT = {"d2": F32, "va": F32R, "r1": F32, "r2": F32}
    lhs_d, rhs_d = {}, {}
    for q in KS:
        lhs_d[q] = nc.dram_tensor(f"lhs_{q}", [KS[q], RPB * 128], MT[q],
                                  kind="ExternalInput")
        rhs_d[q] = nc.dram_tensor(f"rhs_{q}", [KS[q], EXT], MT[q],
                                  kind="ExternalInput")
    out_d = nc.dram_tensor("out", [128, NCOL], F32, kind="ExternalOutput")

    with tile.TileContext(nc) as tc:
        with (
            tc.tile_pool(name="io", bufs=1) as io,
            tc.tile_pool(name="wk", bufs=2) as wk,
            tc.tile_pool(name="ps", bufs=4, space="PSUM") as ps,
        ):
            lhs_s, rhs_s = {}, {}
            for q in KS:
                lhs_s[q] = io.tile([KS[q], RPB * 128], MT[q], name=f"lhs_{q}")
                nc.sync.dma_start(lhs_s[q][:], lhs_d[q][:])
                rhs_s[q] = io.tile([KS[q], EXT], MT[q], name=f"rhs_{q}")
                nc.sync.dma_start(rhs_s[q][:], rhs_d[q][:])
            ocols = io.tile([128, NCOL], F32)

            from contextlib import nullcontext
            loop_cm = tc.For_i(0, reps, 1) if reps > 1 else nullcontext()
            with loop_cm:
              for it in range(RPB):
                diag = False
                p = {}
                for q in KS:
                    p[q] = ps.tile([128, FD], F32, name=f"p_{q}", tag="mm")
                    if True:
                        # block index within the core is `it`; the absolute
                        # block is 8*half+it, but lhs is core-local. Columns
                        # start at 128*(abs_blk+1); half is baked on host by
                        # shifting rhs (see make_in_maps) so here we use a
                        # per-core uniform c0 = 128*(it+1) into the shifted rhs.
                        c0 = 128 * (it + 1)
                        for h in range(2):
                            nc.tensor.matmul(
                                p[q][:, h * 512:(h + 1) * 512],
                                lhs_s[q][:, it * 128:(it + 1) * 128],
                                rhs_s[q][:, c0 + h * 512:c0 + (h + 1) * 512],
                                start=True, stop=True,
                            )

                r1t = wk.tile([128, FD], F16, tag="r1t")
                nc.scalar.activation(r1t[:], p["r1"][:], AF.Sqrt)
                r2t = wk.tile([128, FD], F16, tag="r2t")
                nc.scalar.activation(r2t[:], p["r2"][:], AF.Sqrt)
                dst = wk.tile([128, FD], F16, tag="dst")
                nc.scalar.activation(dst[:], p["d2"][:], AF.Sqrt)
                ht = wk.tile([128, FD], F32, tag="ht")
                nc.vector.reciprocal_approx_fast(ht[:], p["d2"][:])
                invt = wk.tile([128, FD], F16, tag="invt")
                nc.scalar.activation(invt[:], ht[:], AF.Sqrt)
                rva = wk.tile([128, FD], F16, tag="rva")
                eng("rva").tensor_scalar(rva[:], p["va"][:], 0.1, 0.0,
                                         ALU.mult, ALU.max)

                rsum = wk.tile([128, FD], F16, tag="rsum")
                eng("rsum").tensor_add(rsum[:], r1t[:], r2t[:])
                mnum = wk.tile([128, FD], F16, tag="mnum")
                eng("mnum").tensor_mul(mnum[:], rsum[:], invt[:])
                ovi = wk.tile([128, FD], F16, tag="ovi")
                eng("ovi").tensor_sub(ovi[:], mnum[:], dst[:])

                # accumulating ops, split by weight slice
                # (lo, hi, u_col, ru_col, vt_col)
                b0 = 6 * it
                slcs = [(0, W2, b0, b0 + 2, b0 + 4),
                        (W2, FD, b0 + 1, b0 + 3, b0 + 5)]
                # accum semantics: out = in0 op0 s1; accum = reduce(out, op1, init=s2)
                ut = wk.tile([128, FD], F32, tag="ut")
                eng("u").tensor_scalar_add(ut[:], ovi[:], 10.0)
                scr3 = wk.tile([128, FD], F16, tag="scr3")
                for (lo, hi, uc, rc, vc) in slcs:
                    eng("u").tensor_scalar(
                        scr3[:, lo:hi], ovi[:, lo:hi], 0.0, 0.0,
                        ALU.max, ALU.add,
                        accum_out=ocols[:, uc:uc + 1])
                rut = wk.tile([128, FD], F32, tag="ut")
                nc.vector.reciprocal_approx_fast(rut[:], ut[:])
                scr = wk.tile([128, FD], F16, tag="scr")
                for (lo, hi, uc, rc, vc) in slcs:
                    eng("ruS").tensor_scalar(
                        scr[:, lo:hi], rut[:, lo:hi], 0.1, 0.0,
                        ALU.min, ALU.add,
                        accum_out=ocols[:, rc:rc + 1])

                m1 = wk.tile([128, FD], F16, tag="m1")
                eng("m1").tensor_mul(m1[:], rva[:], invt[:])
                pv = wk.tile([128, FD], F16, tag="pv")
                eng("pv").tensor_mul(pv[:], ovi[:], m1[:])
                scr2 = wk.tile([128, FD], F16, tag="scr2")
                for (lo, hi, uc, rc, vc) in slcs:
                    eng("vt").tensor_scalar(
                        scr2[:, lo:hi], pv[:, lo:hi], 0.0, 0.0,
                        ALU.max, ALU.add,
                        accum_out=ocols[:, vc:vc + 1])

            nc.sync.dma_start(out_d[:], ocols[:])

    nc.compile()
    _NC_CACHE[key] = nc
    return nc


def make_in_maps(xyz, scales, rotations, velocities):
    lhs, rhs = _prep(xyz, scales, rotations, velocities)
    in_maps = []
    for c in range(NC):
        b, half = c // 2, c % 2
        m = {}
        for q in lhs:
            m[f"lhs_{q}"] = np.ascontiguousarray(
                lhs[q][b][:, half * 1024:(half + 1) * 1024])
            # shift rhs so core-local column math is uniform: this core's
            # block `it` (absolute 8*half+it) needs cols starting at
            # 128*(8*half+it+1); baking the 1024*half shift here lets the
            # device use c0 = 128*(it+1). Diag pass needs cols at
            # 128*(8*half+s) = shifted 128*s.  Shift = roll left by 1024*half.
            r = rhs[q][b]
            if half:
                r = np.roll(r, -1024, axis=1)
            m[f"rhs_{q}"] = np.ascontiguousarray(r)
        in_maps.append(m)
    return in_maps


_DIAG = {"v": None}


def _host_diag(xyz, scales, rotations, velocities):
    """fp64 sums over the 16*B diagonal 128x128 blocks (O(N*128) pairs)."""
    x = xyz.astype(np.float64)
    s = scales.astype(np.float64)
    v = velocities.astype(np.float64)
    R = _quat_to_rotmat(rotations.astype(np.float64))
    Su = Sru = Svt = 0.0
    for b in range(B):
        for blk in range(NB):
            sl = slice(blk * 128, (blk + 1) * 128)
            xb, sb, vb, Rb = x[b, sl], s[b, sl], v[b, sl], R[b, sl]
            diff = xb[:, None, :] - xb[None, :, :]
            d2 = (diff * diff).sum(-1) + CL
            dist = np.sqrt(d2)
            inv = 1.0 / dist
            dR1 = np.einsum("nmi,nij->nmj", diff, Rb)
            r1s = ((sb[None, :, :] ** 2) * dR1 ** 2).sum(-1) + EPS
            dR2 = np.einsum("nmi,mij->nmj", diff, Rb)
            r2s = ((sb[:, None, :] ** 2) * dR2 ** 2).sum(-1) + EPS
            ovi = (np.sqrt(r1s) + np.sqrt(r2s)) * inv - dist
            np.fill_diagonal(ovi, 0.0)
            nva = -np.einsum("nmi,nmi->nm", vb[:, None, :] - vb[None, :, :], diff)
            rva = np.maximum(nva * 0.1, 0.0)
            Su += np.maximum(ovi, 0.0).sum()
            Sru += np.minimum(1.0 / (ovi + 10.0), 0.1).sum()
            Svt += np.maximum(ovi * rva * inv, 0.0).sum()
    return Su, Sru, Svt


def finish(results):
    Su, Sru, Svt = _DIAG["v"]
    for c in range(NC):
        O = results[c]["out"].astype(np.float64)
        cs = O.sum(axis=0)                                  # (NCOL,)
        w2 = cs[0:48:6].sum(), cs[2:48:6].sum(), cs[4:48:6].sum()
        w1 = cs[1:48:6].sum(), cs[3:48:6].sum(), cs[5:48:6].sum()
        Su += 2 * w2[0] + w1[0]
        Sru += 2 * w2[1] + w1[1]
        Svt += 2 * w2[2] + w1[2]
    tot = float(B) * N * N
    spec = 10.0 * Su - 100.0 * tot + 1000.0 * Sru
    return np.float32(spec / tot + Svt / tot)


_RUNNER = {}


def _get_runner(reps=1):
    """Cached shard_map-jitted executor (mirrors bass2jax.run_bass_via_pjrt
    multi-core path) so repeated calls skip re-compilation."""
    if reps in _RUNNER:
        return _RUNNER[reps]
    import jax
    from jax.sharding import Mesh, PartitionSpec
    from jax.experimental.shard_map import shard_map
    from concourse import bass2jax

    nc = _build(reps)
    bass2jax.install_neuronx_cc_hook()

    part_name = nc.partition_id_tensor.name if nc.partition_id_tensor else None
    in_names, out_names, out_avals, zero_outs = [], [], [], []
    for alloc in nc.m.functions[0].allocations:
        if not isinstance(alloc, mybir.MemoryLocationSet):
            continue
        name = alloc.memorylocations[0].name
        if alloc.kind == "ExternalInput":
            if name != part_name:
                in_names.append(name)
        elif alloc.kind == "ExternalOutput":
            out_names.append(name)
            shape = tuple(alloc.tensor_shape)
            dtype = mybir.dt.np(alloc.dtype)
            out_avals.append(jax.core.ShapedArray(shape, dtype))
            zero_outs.append(np.zeros(shape, dtype))
    n_params = len(in_names)
    all_names = in_names + out_names
    if part_name is not None:
        all_names = all_names + [part_name]

    def _body(*args):
        operands = list(args)
        if part_name is not None:
            operands.append(bass2jax.partition_id_tensor())
        outs = bass2jax._bass_exec_p.bind(
            *operands,
            out_avals=tuple(out_avals),
            in_names=tuple(all_names),
            out_names=tuple(out_names),
            lowering_input_output_aliases=(),
            sim_require_finite=True,
            sim_require_nnan=True,
            nc=nc,
        )
        return tuple(outs)

    devices = jax.devices()[:NC]
    mesh = Mesh(np.asarray(devices), ("core",))
    n_outs = len(out_names)
    fn = jax.jit(
        shard_map(
            _body, mesh=mesh,
            in_specs=(PartitionSpec("core"),) * (n_params + n_outs),
            out_specs=(PartitionSpec("core"),) * n_outs,
            check_rep=False,
        ),
        donate_argnums=tuple(range(n_params, n_params + n_outs)),
        keep_unused=True,
    )

    def run(in_maps):
        concat_in = [
            np.concatenate([in_maps[c][nm] for c in range(NC)], axis=0)
            for nm in in_names
        ]
        concat_zeros = [
            np.zeros((NC * z.shape[0], *z.shape[1:]), z.dtype) for z in zero_outs
        ]
        out_arrs = fn(*concat_in, *concat_zeros)
        return [
            {nm: np.asarray(out_arrs[i]).reshape(NC, *out_avals[i].shape)[c]
             for i, nm in enumerate(out_names)}
            for c in range(NC)
        ]

    _RUNNER[reps] = run
    return run


def kernel(xyz, scales, rotations, velocities):
    run = _get_runner()
    in_maps = make_in_maps(xyz, scales, rotations, velocities)
    _DIAG["v"] = _host_diag(xyz, scales, rotations, velocities)
    return finish(run(in_maps))


if __name__ == "__main__":
    rng = np.random.default_rng(0)
    ins = {
        "xyz": rng.standard_normal((B, N, 3)).astype(np.float32),
        "scales": rng.random((B, N, 3)).astype(np.float32),
        "rotations": rng.standard_normal((B, N, 4)).astype(np.float32),
        "velocities": rng.standard_normal((B, N, 3)).astype(np.float32),
    }
    print(kernel(**ins))
